# revision 2
# baseline (speedup 1.0000x reference)
"""Grouped-experts SwiGLU MoE kernel for 8 Trainium2 NeuronCores.

Problem: x[16384, 2048] routed to 64 experts (256 contiguous tokens each);
per expert e: out_e = (silu(x_e @ w1[e]) * (x_e @ w3[e])) @ w2[e].

Sharding: expert-parallel. Core c owns experts 8c..8c+7 and tokens
[2048c, 2048(c+1)); tokens are pre-permuted so each core computes its own
token slice fully locally (no all-to-all).

This is a memory-bound kernel (each weight element is used exactly once).
All streamed tensors are bf16 (host-converted): 12 MiB weights + 1 MiB xT
in, 1 MiB out per expert -> 112 MiB per core, ~= the bf16 compute roofline.

Per-core device program, per expert (8 per core):
  stage 1 (m-outer): for each 128-wide hidden block m, accumulate
    g|u [128 hid, 256 tok] over 16 dim k-tiles into ONE psum bank
    (w1/w3 tiles stationary, xT moving); then silu (ACT) * u (DVE) ->
    hT[m] bf16 in SBUF.  Only ~2 psum banks live at a time.
  stage 2: out[tok, dim] = h @ w2 via (hT stationary, w2 moving), n-major,
    8 hidden k2-tiles accumulated per [128, 512] psum bank.
x is pre-transposed on host (xT[dim, tok] per expert) so the device does
zero transposes.  All weight/x DMAs are per-partition-contiguous 8-16 KiB
runs (host repack).  Output is written bf16 and upcast on host.
"""

import numpy as np
import ml_dtypes

import concourse.bacc as bacc
import concourse.mybir as mybir
from concourse.bass_utils import run_bass_kernel_spmd
from concourse.tile import TileContext

BF16NP = ml_dtypes.bfloat16

N_CORES = 8
E_PER_CORE = 8          # experts per core
TOK = 256               # tokens per expert
DIM = 2048
HID = 1024
P = 128
KT = DIM // P           # 16 k-tiles (contraction over dim, stage 1)
KT2 = HID // P          # 8 k-tiles (contraction over hidden, stage 2)
MT = HID // P           # 8 hidden m-blocks in stage 1
NCH = DIM // 512        # 4 output n-chunks of 512 in stage 2
CH_M = 2                # m-blocks per w13 DMA chunk (2 MiB bf16)
NMB = MT // CH_M        # 4 w13 chunks per expert
CH_N = 2                # n-chunks per w2 DMA chunk (2 MiB bf16)
NNB = NCH // CH_N       # 2 w2 chunks per expert

F32 = mybir.dt.float32
BF16 = mybir.dt.bfloat16
SILU = mybir.ActivationFunctionType.Silu
MULT = mybir.AluOpType.mult

_program_cache = {}


def _build_program():
    """Per-core Bass program. Same program for all 8 cores (SPMD)."""
    nc = bacc.Bacc("TRN2", target_bir_lowering=False, debug=False)

    # xT: row (e, p) -> KT*TOK contiguous bf16 (8 KiB): [k, t] blocks
    xT_d = nc.dram_tensor("xT", [E_PER_CORE * P, KT * TOK], BF16,
                          kind="ExternalInput")
    # w13: row (e, mb, p) -> CH_M*KT*256 contiguous (16 KiB):
    #   [mi, k, c] with c = [w1 128-col block | w3 128-col block]
    w13_d = nc.dram_tensor("w13", [E_PER_CORE * NMB * P, CH_M * KT * 256],
                           BF16, kind="ExternalInput")
    # w2: row (e, nb, p) -> CH_N*KT2*512 contiguous (16 KiB): [ni, k2, c]
    w2_d = nc.dram_tensor("w2p", [E_PER_CORE * NNB * P, CH_N * KT2 * 512],
                          BF16, kind="ExternalInput")
    out_d = nc.dram_tensor("out", [E_PER_CORE * TOK, DIM], BF16,
                           kind="ExternalOutput")

    with TileContext(nc) as tc:
        with tc.tile_pool(name="xT", bufs=2) as xT_p, \
             tc.tile_pool(name="w13", bufs=6) as w13_p, \
             tc.tile_pool(name="w2", bufs=3) as w2_p, \
             tc.tile_pool(name="hT", bufs=18) as hT_p, \
             tc.tile_pool(name="gs", bufs=3) as gs_p, \
             tc.tile_pool(name="osb", bufs=2) as osb_p, \
             tc.tile_pool(name="psg", bufs=3, space="PSUM") as psg_p, \
             tc.tile_pool(name="pso", bufs=3, space="PSUM") as pso_p:

            # Stage-2 of expert e-1 is interleaved into stage-1 of expert e
            # (one 8-MM stage-2 group after each 32-MM stage-1 m-block) so
            # the w13 stream is consumed at a flat ~190 GB/s instead of
            # 286 GB/s bursts — the prefetch buffer then actually fills and
            # absorbs HBM jitter (the stalls seen at ~96% HBM utilization).
            def w2_fetch(e1, nb):
                w2t = w2_p.tile([P, CH_N, KT2, 512], BF16, tag="w2")
                row0 = (e1 * NNB + nb) * P
                nc.scalar.dma_start(
                    out=w2t[:],
                    in_=w2_d[row0:row0 + P, :].rearrange(
                        "p (n k c) -> p n k c", n=CH_N, k=KT2),
                )
                return w2t

            def s2_group(ctx2, idx):
                hT_prev, osb, w2ts, e1 = ctx2
                nb, ni, m2 = idx // 4, (idx % 4) // 2, idx % 2
                w2t = w2ts[nb]
                n = nb * CH_N + ni
                ops = pso_p.tile([P, 512], F32, tag="ops")
                for k2 in range(KT2):
                    nc.tensor.matmul(
                        ops[:],
                        lhsT=hT_prev[k2][:, m2 * P:(m2 + 1) * P],
                        rhs=w2t[:, ni, k2, :],
                        start=(k2 == 0), stop=(k2 == KT2 - 1))
                nc.vector.tensor_copy(
                    osb[:, m2, n * 512:(n + 1) * 512], ops[:])
                if m2 == 1:
                    out_dst = out_d[e1 * TOK:(e1 + 1) * TOK, :].rearrange(
                        "(m p) c -> p m c", p=P)
                    nc.scalar.dma_start(
                        out=out_dst[:, :, n * 512:(n + 1) * 512],
                        in_=osb[:, :, n * 512:(n + 1) * 512],
                    )

            prev = None
            for e in range(E_PER_CORE):
                # ---- xT for this expert: [128, k, t] ----
                xt = xT_p.tile([P, KT, TOK], BF16, tag="xT")
                xsrc = xT_d[e * P:(e + 1) * P, :].rearrange(
                    "p (k t) -> p k t", k=KT)
                if e == 0:
                    # halve the first-matmul DMA gate at kernel start
                    nc.scalar.dma_start(out=xt[:, 0:KT // 2], in_=xsrc[:, 0:KT // 2])
                    nc.scalar.dma_start(out=xt[:, KT // 2:], in_=xsrc[:, KT // 2:])
                else:
                    nc.scalar.dma_start(out=xt[:], in_=xsrc)
                # prefetch this expert's first w2 chunk a full stage-1 early
                w2_nb0 = w2_fetch(e, 0)

                # ---- stage 1 (m-outer) with stage-2(e-1) interleaved ----
                hT = []
                for mb in range(NMB):
                    wt = w13_p.tile([P, CH_M, KT, 256], BF16, tag="w13")
                    row0 = (e * NMB + mb) * P
                    src = w13_d[row0:row0 + P, :].rearrange(
                        "p (m k c) -> p m k c", m=CH_M, k=KT)
                    if e == 0:
                        # smaller initial DMA gate at kernel start
                        for mi in range(CH_M):
                            nc.sync.dma_start(out=wt[:, mi], in_=src[:, mi])
                    else:
                        nc.sync.dma_start(out=wt[:], in_=src)
                    for mi in range(CH_M):
                        m = mb * CH_M + mi
                        gu = psg_p.tile([P, 512], F32, tag="gu")
                        for k in range(KT):
                            # start=True clears has_written for the WHOLE
                            # bank; u's first matmul overwrites via
                            # has_written=0 (see PSUM accumulate rules).
                            nc.tensor.matmul(
                                gu[:, 0:TOK],
                                lhsT=wt[:, mi, k, 0:P],
                                rhs=xt[:, k, :], start=(k == 0),
                                stop=(k == KT - 1), skip_group_check=True)
                            nc.tensor.matmul(
                                gu[:, TOK:2 * TOK],
                                lhsT=wt[:, mi, k, P:2 * P],
                                rhs=xt[:, k, :], start=False,
                                stop=(k == KT - 1), skip_group_check=True)
                        gs = gs_p.tile([P, TOK], F32, tag="gs")
                        nc.scalar.activation(gs[:], gu[:, 0:TOK], SILU)
                        ht = hT_p.tile([P, TOK], BF16, tag="hT")
                        hT.append(ht)
                        nc.vector.tensor_tensor(ht[:], gs[:],
                                                gu[:, TOK:2 * TOK], MULT)
                        if prev is not None:
                            s2_group(prev, m)

                osb = osb_p.tile([P, 2, DIM], BF16, tag="osb")
                prev = (hT, osb, {0: w2_nb0, 1: w2_fetch(e, 1)}, e)

            # tail expert's stage 2 (not interleaved with anything)
            for idx in range(8):
                s2_group(prev, idx)

    nc.compile()
    return nc


def _get_program():
    if "nc" not in _program_cache:
        _program_cache["nc"] = _build_program()
    return _program_cache["nc"]


def _prepare_in_maps(x, w1, w2, w3):
    """Host repack: bf16-convert + lay out so every DMA row is contiguous."""
    E = w1.shape[0]
    assert E == N_CORES * E_PER_CORE and x.shape == (E * TOK, DIM)

    xb = np.asarray(x, dtype=np.float32).astype(BF16NP)
    w1b = np.asarray(w1, dtype=np.float32).astype(BF16NP)
    w2b = np.asarray(w2, dtype=np.float32).astype(BF16NP)
    w3b = np.asarray(w3, dtype=np.float32).astype(BF16NP)

    # xT[e, p, k, t] = x[e*TOK + t, k*P + p]
    xT = np.ascontiguousarray(
        xb.reshape(E, TOK, KT, P).transpose(0, 3, 2, 1))

    # w13[e, mb, p, mi, k, c]: c 0:128 = w1[e, kP+p, (mb*CH_M+mi)P + c],
    #                          c 128:256 = same from w3
    w1r = w1b.reshape(E, KT, P, MT, P)
    w3r = w3b.reshape(E, KT, P, MT, P)
    w13 = np.concatenate([w1r, w3r], axis=4)          # [e, k, p, m, 256]
    w13 = w13.transpose(0, 3, 2, 1, 4)                # [e, m, p, k, 256]
    w13 = w13.reshape(E, NMB, CH_M, P, KT, 256).transpose(0, 1, 3, 2, 4, 5)
    w13 = np.ascontiguousarray(w13)                   # [e, mb, p, mi, k, c]

    # w2p[e, nb, p, ni, k2, c] = w2[e, k2*P + p, (nb*CH_N+ni)*512 + c]
    w2r = w2b.reshape(E, KT2, P, NCH, 512).transpose(0, 3, 2, 1, 4)
    w2p = w2r.reshape(E, NNB, CH_N, P, KT2, 512).transpose(0, 1, 3, 2, 4, 5)
    w2p = np.ascontiguousarray(w2p)                   # [e, nb, p, ni, k2, c]

    in_maps = []
    for c in range(N_CORES):
        e0 = c * E_PER_CORE
        in_maps.append({
            "xT": xT[e0:e0 + E_PER_CORE].reshape(E_PER_CORE * P, KT * TOK),
            "w13": w13[e0:e0 + E_PER_CORE].reshape(E_PER_CORE * NMB * P,
                                                   CH_M * KT * 256),
            "w2p": w2p[e0:e0 + E_PER_CORE].reshape(E_PER_CORE * NNB * P,
                                                   CH_N * KT2 * 512),
        })
    return in_maps


def kernel(x, w1, w2, w3, num_local_tokens_per_expert=None, **_unused):
    in_maps = _prepare_in_maps(x, w1, w2, w3)
    nc = _get_program()
    res = run_bass_kernel_spmd(nc, in_maps, list(range(N_CORES)))
    return np.concatenate(
        [np.asarray(res.results[c]["out"]).astype(np.float32)
         for c in range(N_CORES)], axis=0)
